# revision 1
# baseline (speedup 1.0000x reference)
"""
MoE-routing kernel for Trainium2 (8 NeuronCores, SPMD via bass).

Computation (matches the reference):
  attended[b, c] = sum_hw((mask[b, hw] + 1e-10) * feat[b, c, hw]) / sum_hw(mask[b, hw] + 1e-10)
  out[b, a]      = attended[b, :] @ W[inst[b], a, :] + bias[inst[b], a]

Strategy: split the channel dim C=2048 into 8 shards of 256 (one per core).
Each core computes a partial contraction over its channel shard for ALL 256
samples; the host sums the 8 partials.  The batch is sorted by expert on the
host (static routing baked into the compiled program), so each expert's
samples form a contiguous group of stationary columns for the grouped GEMM.

Per core:
  phase 1 (pooling): for each sample, PE broadcasts the mask row to 128
    partitions (K=1 matmul), DVE does a fused multiply+reduce
    (tensor_tensor_reduce) against the feature tile -> one column of
    attended^T per (sample, c-tile).  Unnormalized (raw mask).
  phase 2 (grouped GEMM): per expert group, stationary = attended^T columns
    of the group, moving = W^T [c, a] chunks streamed from HBM (float32r ->
    full PE rate).  An extra K=1 matmul accumulates msum[b] * bias[e, a]
    into PSUM; eviction multiplies rows by 1/msum[b] (per-partition scalar),
    which normalizes the pooled features and leaves bias intact.
"""

import sys

if "/opt/trn_rl_repo" not in sys.path:
    sys.path.insert(0, "/opt/trn_rl_repo")

import numpy as np

import concourse.bass as bass
import concourse.mybir as mybir
import concourse.tile as tile
from concourse import bacc
from concourse import bass_utils
from concourse.masks import make_identity

# Problem constants (hardcoded; kernel.py must be self-contained)
B = 256          # batch
C = 2048         # channels
HW = 196         # spatial positions (14*14)
E = 16           # experts
A = 3000         # answers
NCORES = 8
CS = C // NCORES  # channel shard per core = 256
P = 128
KT = CS // P      # k-tiles per core = 2
MROW_BATCH = 16   # samples per partition-0 mask-row tile
HWP = 256         # padded mask row width (f32r wants moving free >= 256)
CHUNKS = [(c0, min(512, A - c0)) for c0 in range(0, A, 512)]

F32 = mybir.dt.float32
F32R = mybir.dt.float32r


def _make_groups(counts):
    """[(gstart_in_sorted_order, gsz, expert)] with gsz <= 128."""
    groups = []
    start = 0
    for e in range(E):
        n = int(counts[e])
        g0 = start
        while n > 0:
            gsz = min(n, P)
            groups.append((g0, gsz, e))
            g0 += gsz
            n -= gsz
        start += int(counts[e])
    return groups


def build_program(groups, loop_n=1, do_pool=True, do_mm=True, do_evict=True, pool_mode='full'):
    """Build + compile the per-core Bass program (identical on all cores)."""
    nc = bacc.Bacc("TRN2", target_bir_lowering=False, debug=False,
                   num_devices=NCORES)

    feat_d = nc.dram_tensor("feat", [B, CS, HW], F32, kind="ExternalInput").ap()
    mask_d = nc.dram_tensor("mask", [B, HW], F32, kind="ExternalInput").ap()
    wt_d = nc.dram_tensor("wt", [E, CS, A], F32R, kind="ExternalInput").ap()
    bias_d = nc.dram_tensor("bias", [1, E * A], F32R, kind="ExternalInput").ap()
    part_d = nc.dram_tensor("part", [B, A], F32, kind="ExternalOutput").ap()

    import contextlib
    with tile.TileContext(nc) as tc:
        loop_ctx = tc.For_i(0, loop_n, 1) if loop_n > 1 else contextlib.nullcontext()
        with (
            loop_ctx,
            tc.tile_pool(name="persist", bufs=1) as pp,
            tc.tile_pool(name="feat", bufs=6) as fp,
            tc.tile_pool(name="mrow", bufs=3) as mrp,
            tc.tile_pool(name="wt", bufs=6) as wtp,
            tc.tile_pool(name="bias", bufs=2) as bp,
            tc.tile_pool(name="outs", bufs=4) as op,
            tc.tile_pool(name="bcast", bufs=4) as pbc,
            tc.tile_pool(name="ps_mm", bufs=3, space="PSUM") as pmm,
            tc.tile_pool(name="ps_sm", bufs=2, space="PSUM") as psm,
        ):
            # ---- constants ----
            ident = pp.tile([P, P], F32, tag="ident")
            make_identity(nc, ident)
            ones32 = pp.tile([1, 1], F32, tag="ones32")
            nc.vector.memset(ones32, 1.0)
            dummy = pp.tile([P, 1], F32, tag="dummy")

            # ---- mask: per-sample sums and reciprocals ----
            mbp = pp.tile([P, KT, HW], F32, tag="mbp")
            nc.sync.dma_start(mbp, mask_d.rearrange("(t p) f -> p t f", p=P))
            msum = pp.tile([P, KT], F32, tag="msum")
            nc.vector.tensor_reduce(msum, mbp,
                                    axis=mybir.AxisListType.X,
                                    op=mybir.AluOpType.add)
            nc.vector.tensor_scalar_add(msum, msum, HW * 1e-10)

            # msum as a partition-0 row [1, B] (exact fp32 extraction matmuls)
            msum_row = pp.tile([1, B], F32R, tag="msum_row")
            for t in range(KT):
                pt = psm.tile([1, P], F32, name="pt_row", tag="pt")
                nc.tensor.matmul(pt, lhsT=msum[:, t:t + 1], rhs=ident,
                                 start=True, stop=True)
                nc.vector.tensor_copy(msum_row[0:1, t * P:(t + 1) * P], pt)
            recip_row = pp.tile([1, B], F32, tag="recip_row")
            nc.vector.reciprocal(recip_row, msum_row)

            # per-group reciprocals at partition base 0: rg[r] = 1/msum[g0+r]
            rgrps = []
            for gi, (g0, gsz, e) in enumerate(groups):
                rg = pp.tile([P, 1], F32, tag=f"rgrp{gi}", name=f"rgrp{gi}")
                pt = psm.tile([P, 1], F32, name="pt_col", tag="pt")
                nc.tensor.matmul(pt[:gsz], lhsT=recip_row[0:1, g0:g0 + gsz],
                                 rhs=ones32[0:1, 0:1], start=True, stop=True)
                nc.vector.tensor_copy(rg[:gsz], pt[:gsz])
                rgrps.append(rg)

            # mask rows on partition 0 for the PE broadcast (batched loads)
            mrows = []
            for mb in range(B // MROW_BATCH):
                mt = mrp.tile([1, MROW_BATCH, HW], F32, tag="mrow")
                nc.sync.dma_start(
                    mt, mask_d[mb * MROW_BATCH:(mb + 1) * MROW_BATCH, :]
                    .rearrange("(o s) f -> o s f", o=1))
                mrows.append(mt)

            # attended^T tiles, one per group: [128 c, KT, gsz]
            atts = [pp.tile([P, KT, gsz], F32R, tag=f"att{gi}", name=f"att{gi}")
                    for gi, (g0, gsz, e) in enumerate(groups)]
            if not do_pool:
                for att in atts:
                    nc.gpsimd.memset(att.bitcast(F32), 0.0)

            # ---- phase 1: masked pooling, one sample at a time ----
            sample_group = {}
            for gi, (g0, gsz, e) in enumerate(groups):
                for s in range(g0, g0 + gsz):
                    sample_group[s] = (gi, s - g0)
            for s in range(B):
                gi, pos = sample_group[s]
                ft = fp.tile([P, KT, HW], F32, tag="feat")
                nc.sync.dma_start(ft, feat_d[s].rearrange("(t p) f -> p t f", p=P))
                if do_pool:
                    bc = pbc.tile([P, HW], F32, name="bc")
                    if pool_mode in ("full", "bconly"):
                        nc.gpsimd.partition_broadcast(
                            bc, mrows[s // MROW_BATCH][0:1, s % MROW_BATCH])
                    if pool_mode == "full":
                        in1s = [bc] * KT
                    elif pool_mode == "sttsbuf":
                        in1s = [ft[:, t] for t in range(KT)]
                    else:
                        in1s = None
                    if in1s is not None:
                        for t in range(KT):
                            nc.vector.scalar_tensor_tensor(
                                dummy.broadcast_to([P, HW]),
                                ft[:, t], 1.0, in1s[t],
                                op0=mybir.AluOpType.mult, op1=mybir.AluOpType.mult,
                                accum_out=atts[gi][:, t, pos:pos + 1])

            # ---- phase 2: grouped GEMM over answer chunks ----
            bias_tiles = {}
            for gi, (g0, gsz, e) in enumerate(groups):
                if e not in bias_tiles:
                    bt = bp.tile([1, A], F32R, tag="bias")
                    nc.sync.dma_start(bt, bias_d[0:1, e * A:(e + 1) * A])
                    bias_tiles[e] = bt
                bt = bias_tiles[e]
                att = atts[gi]
                for (c0, cw) in CHUNKS:
                    wt = wtp.tile([P, KT, cw], F32R, tag="wt")
                    nc.sync.dma_start(
                        wt, wt_d[e].rearrange("(t p) a -> p t a", p=P)[:, :, c0:c0 + cw])
                    ot = op.tile([P, 512], F32, tag="out")
                    if not do_mm:
                        nc.gpsimd.memset(ot[:gsz, :cw], 0.0)
                    if do_mm:
                        ps = pmm.tile([P, 512], F32, name="ps")
                        for t in range(KT):
                            nc.tensor.matmul(
                                ps[:gsz, :cw],
                                lhsT=att[:, t],
                                rhs=wt[:, t],
                                start=(t == 0), stop=False)
                        nc.tensor.matmul(
                            ps[:gsz, :cw],
                            lhsT=msum_row[0:1, g0:g0 + gsz],
                            rhs=bt[0:1, c0:c0 + cw],
                            start=False, stop=True)
                        if do_evict:
                            nc.vector.tensor_scalar_mul(ot[:gsz, :cw], ps[:gsz, :cw],
                                                        rgrps[gi][:gsz])
                        else:
                            nc.gpsimd.memset(ot[:gsz, :cw], 0.0)
                    nc.sync.dma_start(part_d[g0:g0 + gsz, c0:c0 + cw],
                                      ot[:gsz, :cw])

    nc.compile()
    return nc


_PROGRAM_CACHE = {}


def _get_program(groups):
    key = tuple(groups)
    if key not in _PROGRAM_CACHE:
        _PROGRAM_CACHE[key] = build_program(groups)
    return _PROGRAM_CACHE[key]


def make_in_maps(mask, features, W, b, inst):
    """Host-side routing + sharding.  Returns (in_maps, perm, groups)."""
    inst_np = np.asarray(inst)
    perm = np.argsort(inst_np, kind="stable")
    counts = np.bincount(inst_np.astype(np.int64), minlength=E)
    groups = _make_groups(counts)

    mask_pad = np.ascontiguousarray(np.asarray(mask, np.float32).reshape(B, HW)[perm])

    feat = np.asarray(features, np.float32).reshape(B, C, HW)[perm]
    Wf = np.asarray(W, np.float32)
    bias_row = np.asarray(b, np.float32).reshape(1, E * A)
    zero_bias = np.zeros_like(bias_row)

    in_maps = []
    for k in range(NCORES):
        sl = slice(k * CS, (k + 1) * CS)
        feat_k = np.ascontiguousarray(feat[:, sl])
        wt_k = np.ascontiguousarray(Wf[:, :, sl].transpose(0, 2, 1))
        in_maps.append({
            "feat": feat_k,
            "mask": mask_pad,
            "wt": wt_k,
            "bias": bias_row if k == 0 else zero_bias,
        })
    return in_maps, perm, groups


def postprocess(results, perm):
    part = np.zeros((B, A), np.float32)
    for r in results:
        part += r["part"]
    out = np.empty((B, A), np.float32)
    out[perm] = part
    return out


def kernel(mask, features, W, b, inst):
    in_maps, perm, groups = make_in_maps(mask, features, W, b, inst)
    nc = _get_program(groups)
    res = bass_utils.run_bass_kernel_spmd(nc, in_maps, core_ids=list(range(NCORES)))
    return postprocess(res.results, perm)



# revision 8
# speedup vs baseline: 1.4849x; 1.4849x over previous
"""
MoE-routing kernel for Trainium2 (8 NeuronCores, SPMD via bass).

Computation (matches the reference):
  attended[b, c] = sum_hw(mn[b, hw] * feat[b, c, hw]),  mn = (m+1e-10)/sum(m+1e-10)
  out[b, a]      = attended[b, :] @ W[inst[b], a, :] + bias[inst[b], a]

Strategy: channel-sharded over 8 cores (CS = 2048/8 = 256 channels each);
host sums the 8 partial [B, A] outputs and adds the bias.  Samples are
sorted by expert on the host so each expert's samples form contiguous
stationary columns.  All streamed tensors (feat, mask, W) are cast to
fp16 on the host, halving DMA traffic; accumulations stay fp32 on device.

Per core:
  phase 1 (pooling): samples on partitions.  For each 32-channel chunk,
    DVE multiplies feat[128s, 32c, 196hw] by the normalized mask row
    (broadcast along c) and reduces hw in two stages (fp16 14-wide, then
    fp32) -> att[128s, 256c] fp32.
  phase 1.5: PE transposes att into att_T[128c, kt, 256s] (fp16).
  phase 2 (grouped GEMM): per expert group and 512-answer chunk, two
    fp16 matmuls (stationary = att_T columns, moving = W^T chunk)
    accumulate in PSUM; the result DMAs straight from PSUM to HBM.
DMA issue is split: input streams on SP, output drains on Activation.
"""

import sys

if "/opt/trn_rl_repo" not in sys.path:
    sys.path.insert(0, "/opt/trn_rl_repo")

import numpy as np

import concourse.bass as bass
import concourse.mybir as mybir
import concourse.tile as tile
from concourse import bacc
from concourse import bass_utils
from concourse.masks import make_identity

# Problem constants (hardcoded; kernel.py must be self-contained)
B = 256          # batch
C = 2048         # channels
HW = 196         # spatial positions (14*14)
E = 16           # experts
A = 3000         # answers
NCORES = 8
CS = C // NCORES  # channel shard per core = 256
P = 128
KT = CS // P      # k-tiles per core = 2
ST = B // P       # sample tiles = 2
CCH = 32          # channels per pooling chunk
NCH = CS // CCH   # pooling chunks per sample tile = 8
CHUNKS = [(c0, min(512, A - c0)) for c0 in range(0, A, 512)]

F32 = mybir.dt.float32
F16 = mybir.dt.float16


def _make_groups(counts):
    """[(gstart_in_sorted_order, gsz, expert)] with gsz <= 128."""
    groups = []
    start = 0
    for e in range(E):
        n = int(counts[e])
        g0 = start
        while n > 0:
            gsz = min(n, P)
            groups.append((g0, gsz, e))
            g0 += gsz
            n -= gsz
        start += int(counts[e])
    return groups


def build_program(groups, loop_n=1):
    """Build + compile the per-core Bass program (identical on all cores)."""
    nc = bacc.Bacc("TRN2", target_bir_lowering=False, debug=False,
                   num_devices=NCORES)

    feat_d = nc.dram_tensor("feat", [ST, P, CS, HW], F16, kind="ExternalInput").ap()
    mask_d = nc.dram_tensor("mask", [ST, P, HW], F16, kind="ExternalInput").ap()
    wt_d = nc.dram_tensor("wt", [E, KT, P, A], F16, kind="ExternalInput").ap()
    part_d = nc.dram_tensor("part", [B, A], F16, kind="ExternalOutput").ap()

    import contextlib
    with tile.TileContext(nc) as tc:
        loop_ctx = tc.For_i(0, loop_n, 1) if loop_n > 1 else contextlib.nullcontext()
        with (
            loop_ctx,
            tc.tile_pool(name="persist", bufs=1) as pp,
            tc.tile_pool(name="feat", bufs=3) as fp,
            tc.tile_pool(name="tmp", bufs=2) as tp,
            tc.tile_pool(name="red", bufs=2) as rp,
            tc.tile_pool(name="wt", bufs=12) as wtp,
            tc.tile_pool(name="outs", bufs=4) as op,
            tc.tile_pool(name="ps_mm", bufs=3, space="PSUM") as pmm,
            tc.tile_pool(name="ps_tr", bufs=2, space="PSUM") as ptr,
        ):
            ident = pp.tile([P, P], F32, tag="ident")
            make_identity(nc, ident)

            mks = []
            for st in range(ST):
                mk = pp.tile([P, 1, HW], F16, tag=f"mk{st}", name=f"mk{st}")
                nc.sync.dma_start(mk[:, 0, :], mask_d[st])
                mks.append(mk)

            att_s = [pp.tile([P, CS], F32, tag=f"att{st}", name=f"att{st}")
                     for st in range(ST)]
            att_T = pp.tile([P, KT, B], F16, tag="attT")

            # ---- phase 1: masked pooling (samples on partitions) ----
            for st in range(ST):
                for ci in range(NCH):
                    c0 = ci * CCH
                    ft = fp.tile([P, CCH, HW], F16, tag="feat")
                    nc.sync.dma_start(ft, feat_d[st, :, c0:c0 + CCH, :])
                    tm = tp.tile([P, CCH, HW], F16, tag="tmp")
                    nc.vector.tensor_tensor(
                        tm, ft, mks[st].broadcast_to([P, CCH, HW]),
                        op=mybir.AluOpType.mult)
                    t2 = rp.tile([P, CCH, 14], F16, tag="red")
                    with nc.allow_low_precision(reason="fp16 partial reduce"):
                        nc.vector.tensor_reduce(
                            t2, tm.rearrange("p c (u v) -> p c u v", u=14),
                            axis=mybir.AxisListType.X, op=mybir.AluOpType.add)
                    nc.vector.tensor_reduce(
                        att_s[st][:, c0:c0 + CCH], t2,
                        axis=mybir.AxisListType.X, op=mybir.AluOpType.add)

                # transpose into att_T[:, kt, st*128:(st+1)*128], cast at evict
                for t in range(KT):
                    pt = ptr.tile([P, P], F32, name="pt")
                    nc.tensor.transpose(pt, att_s[st][:, t * P:(t + 1) * P], ident)
                    nc.vector.tensor_copy(
                        att_T[:, t, st * P:(st + 1) * P], pt)

            # ---- phase 2: grouped GEMM over answer chunks ----
            for gi, (g0, gsz, e) in enumerate(groups):
                for (c0, cw) in CHUNKS:
                    wt = wtp.tile([P, KT, cw], F16, tag="wt")
                    nc.sync.dma_start(
                        wt, wt_d[e, :, :, c0:c0 + cw].rearrange("t p a -> p t a"))
                    ps = pmm.tile([P, 512], F32, name="ps")
                    for t in range(KT):
                        nc.tensor.matmul(
                            ps[:gsz, :cw],
                            lhsT=att_T[:, t, g0:g0 + gsz],
                            rhs=wt[:, t, :],
                            start=(t == 0), stop=(t == KT - 1))
                    ot = op.tile([P, 512], F16, tag="out")
                    nc.scalar.copy(ot[:gsz, :cw], ps[:gsz, :cw])
                    nc.scalar.dma_start(part_d[g0:g0 + gsz, c0:c0 + cw],
                                        ot[:gsz, :cw])

    nc.compile()
    return nc


_PROGRAM_CACHE = {}


def _get_program(groups):
    key = tuple(groups)
    if key not in _PROGRAM_CACHE:
        _PROGRAM_CACHE[key] = build_program(groups)
    return _PROGRAM_CACHE[key]


def make_in_maps(mask, features, W, b, inst):
    """Host-side routing + sharding.  Returns (in_maps, perm, groups)."""
    inst_np = np.asarray(inst).astype(np.int64)
    perm = np.argsort(inst_np, kind="stable")
    counts = np.bincount(inst_np, minlength=E)
    groups = _make_groups(counts)

    m = np.asarray(mask, np.float64).reshape(B, HW) + 1e-10
    mn = (m / m.sum(1, keepdims=True)).astype(np.float16)
    mask_h = np.ascontiguousarray(mn[perm].reshape(ST, P, HW))

    feat = np.asarray(features, np.float32).reshape(B, C, HW)[perm]
    Wf = np.asarray(W, np.float32)

    in_maps = []
    for k in range(NCORES):
        sl = slice(k * CS, (k + 1) * CS)
        feat_k = np.ascontiguousarray(
            feat[:, sl].reshape(ST, P, CS, HW)).astype(np.float16)
        # wt_k[e, t, p, a] = W[e, a, k*CS + t*128 + p]
        wt_k = np.ascontiguousarray(
            Wf[:, :, sl].transpose(0, 2, 1).reshape(E, KT, P, A)).astype(np.float16)
        in_maps.append({
            "feat": feat_k,
            "mask": mask_h,
            "wt": wt_k,
        })
    return in_maps, perm, groups


def postprocess(results, perm, b, inst):
    part = np.zeros((B, A), np.float32)
    for r in results:
        part += np.asarray(r["part"], np.float32)
    out = np.empty((B, A), np.float32)
    out[perm] = part
    out += np.asarray(b, np.float32)[np.asarray(inst).astype(np.int64)]
    return out


def kernel(mask, features, W, b, inst):
    in_maps, perm, groups = make_in_maps(mask, features, W, b, inst)
    nc = _get_program(groups)
    res = bass_utils.run_bass_kernel_spmd(nc, in_maps, core_ids=list(range(NCORES)))
    return postprocess(res.results, perm, b, inst)


# revision 13
# speedup vs baseline: 5.6250x; 3.7882x over previous
"""
MoE-routing kernel for Trainium2 (8 NeuronCores, SPMD via bass).

Computation (matches the reference):
  attended[b, c] = sum_hw(mn[b, hw] * feat[b, c, hw]),  mn = (m+1e-10)/sum(m+1e-10)
  out[b, a]      = attended[b, :] @ W[inst[b], a, :] + bias[inst[b], a]

Strategy: channel-sharded over 8 cores (CS = 2048/8 = 256 channels each);
host sums the 8 partial [B, A] outputs and adds the bias.  Samples are
sorted by expert on the host so each expert's samples form contiguous
stationary columns.  All streamed tensors (feat, mask, W) are cast to
fp16 on the host, halving DMA traffic; accumulations stay fp32 on device.

Per core:
  phase 1 (pooling on the PE): feat is host-transposed to [s, hw, c] and
    DMA'd 4 samples at a time with hw on partitions (2 k-tiles of 98).
    For each sample and 128-channel tile, matmul(stationary=feat^T[hw,c],
    moving=mn[s] column) contracts hw -- the mask multiply rides inside
    the matmul, so no DVE work and the result lands directly in att^T
    [c, s] layout (psum columns, evicted in 64-sample blocks as fp16).
  phase 2 (grouped GEMM): per expert, one whole-weight DMA [128, KT, A]
    (6 KB descriptors); per group and 512-answer chunk, two fp16 matmuls
    accumulate in PSUM; Activation engine evicts to an SBUF row tile
    which DMAs out once per group (6 KB descriptors).
DMA queues: feat+mask+out on SP HWDGE, weights on Activation HWDGE.
"""

import sys

if "/opt/trn_rl_repo" not in sys.path:
    sys.path.insert(0, "/opt/trn_rl_repo")

import numpy as np

import concourse.bass as bass
import concourse.mybir as mybir
import concourse.tile as tile
from concourse import bacc
from concourse import bass_utils

# Problem constants (hardcoded; kernel.py must be self-contained)
B = 256          # batch
C = 2048         # channels
HW = 196         # spatial positions (14*14)
E = 16           # experts
A = 3000         # answers
NCORES = 8
CS = C // NCORES  # channel shard per core = 256
P = 128
KT = CS // P      # c k-tiles per core = 2
HWT = 2           # hw k-tiles
HWP = HW // HWT   # hw partitions per k-tile = 98
SB = 4            # samples per feat DMA
BLK = 64          # samples per psum evict block
CHUNKS = [(c0, min(512, A - c0)) for c0 in range(0, A, 512)]

F32 = mybir.dt.float32
F16 = mybir.dt.float16


def _make_groups(counts):
    """[(gstart_in_sorted_order, gsz, expert)] with gsz <= 128."""
    groups = []
    start = 0
    for e in range(E):
        n = int(counts[e])
        g0 = start
        while n > 0:
            gsz = min(n, P)
            groups.append((g0, gsz, e))
            g0 += gsz
            n -= gsz
        start += int(counts[e])
    return groups


def build_program(groups, loop_n=1, do_pool=True, do_mm=True):
    """Build + compile the per-core Bass program (identical on all cores)."""
    nc = bacc.Bacc("TRN2", target_bir_lowering=False, debug=False,
                   num_devices=NCORES)

    feat_d = nc.dram_tensor("feat", [B // SB, HWP, SB, HWT, CS], F16,
                            kind="ExternalInput").ap()
    mask_d = nc.dram_tensor("mask", [HWP, HWT, B], F16, kind="ExternalInput").ap()
    wt_d = nc.dram_tensor("wt", [E, KT, P, A], F16, kind="ExternalInput").ap()
    part_d = nc.dram_tensor("part", [B, A], F16, kind="ExternalOutput").ap()

    import contextlib
    with tile.TileContext(nc) as tc:
        loop_ctx = tc.For_i(0, loop_n, 1) if loop_n > 1 else contextlib.nullcontext()
        with (
            loop_ctx,
            tc.tile_pool(name="persist", bufs=1) as pp,
            tc.tile_pool(name="feat", bufs=6) as fp,
            tc.tile_pool(name="wt", bufs=3) as wtp,
            tc.tile_pool(name="outs", bufs=2) as op,
            tc.tile_pool(name="ps_mm", bufs=3, space="PSUM") as pmm,
            tc.tile_pool(name="ps_pool", bufs=2, space="PSUM") as psp,
        ):
            mk = pp.tile([HWP, HWT, B], F16, tag="mask")
            nc.sync.dma_start(mk, mask_d)

            att_T = pp.tile([P, KT, B], F16, tag="attT")
            if not do_pool:
                nc.vector.memset(att_T.bitcast(F32), 0.0)

            # ---- phase 1: pooling on the PE ----
            if do_pool:
                for blk in range(B // BLK):
                    pss = psp.tile([P, KT, BLK], F32, name="psp")
                    for j in range(BLK // SB):
                        sb = blk * (BLK // SB) + j
                        ft = fp.tile([HWP, SB, HWT, CS], F16, tag="feat")
                        nc.sync.dma_start(ft, feat_d[sb])
                        for i in range(SB):
                            s = sb * SB + i
                            pos = j * SB + i
                            for ct in range(KT):
                                for t in range(HWT):
                                    nc.tensor.matmul(
                                        pss[:, ct, pos:pos + 1],
                                        lhsT=ft[:, i, t, ct * P:(ct + 1) * P],
                                        rhs=mk[:, t, s:s + 1],
                                        start=(t == 0), stop=(t == HWT - 1))
                    nc.vector.tensor_copy(
                        att_T[:, :, blk * BLK:(blk + 1) * BLK], pss)

            # ---- phase 2: grouped GEMM, whole-expert weight loads ----
            for gi, (g0, gsz, e) in enumerate(groups):
                wt = wtp.tile([P, KT, A], F16, tag="wt")
                nc.scalar.dma_start(wt, wt_d[e].rearrange("t p a -> p t a"))
                ot = op.tile([P, A], F16, tag="out")
                for (c0, cw) in CHUNKS:
                    if not do_mm:
                        continue
                    ps = pmm.tile([P, 512], F32, name="ps")
                    for t in range(KT):
                        nc.tensor.matmul(
                            ps[:gsz, :cw],
                            lhsT=att_T[:, t, g0:g0 + gsz],
                            rhs=wt[:, t, c0:c0 + cw],
                            start=(t == 0), stop=(t == KT - 1))
                    nc.scalar.copy(ot[:gsz, c0:c0 + cw], ps[:gsz, :cw])
                if not do_mm:
                    nc.vector.memset(ot[:gsz].bitcast(F32), 0.0)
                nc.sync.dma_start(part_d[g0:g0 + gsz, :], ot[:gsz])

    nc.compile()
    return nc


_PROGRAM_CACHE = {}


def _get_program(groups):
    key = tuple(groups)
    if key not in _PROGRAM_CACHE:
        _PROGRAM_CACHE[key] = build_program(groups)
    return _PROGRAM_CACHE[key]


def make_in_maps(mask, features, W, b, inst):
    """Host-side routing + sharding.  Returns (in_maps, perm, groups)."""
    inst_np = np.asarray(inst).astype(np.int64)
    perm = np.argsort(inst_np, kind="stable")
    counts = np.bincount(inst_np, minlength=E)
    groups = _make_groups(counts)

    m = np.asarray(mask, np.float64).reshape(B, HW) + 1e-10
    mn = (m / m.sum(1, keepdims=True)).astype(np.float16)[perm]
    # mask_h[p, t, s] = mn[s, t*HWP + p]
    mask_h = np.ascontiguousarray(mn.reshape(B, HWT, HWP).transpose(2, 1, 0))

    feat = np.asarray(features, np.float32).reshape(B, C, HW)[perm]
    Wf = np.asarray(W, np.float32)

    in_maps = []
    for k in range(NCORES):
        sl = slice(k * CS, (k + 1) * CS)
        # feat_k[sb, p, i, t, c] = feat[sb*SB+i, c_k, t*HWP + p]
        feat_k = np.ascontiguousarray(
            feat[:, sl].reshape(B // SB, SB, CS, HWT, HWP)
            .transpose(0, 4, 1, 3, 2)).astype(np.float16)
        # wt_k[e, t, p, a] = W[e, a, k*CS + t*128 + p]
        wt_k = np.ascontiguousarray(
            Wf[:, :, sl].transpose(0, 2, 1).reshape(E, KT, P, A)).astype(np.float16)
        in_maps.append({
            "feat": feat_k,
            "mask": mask_h,
            "wt": wt_k,
        })
    return in_maps, perm, groups


def postprocess(results, perm, b, inst):
    part = np.zeros((B, A), np.float32)
    for r in results:
        part += np.asarray(r["part"], np.float32)
    out = np.empty((B, A), np.float32)
    out[perm] = part
    out += np.asarray(b, np.float32)[np.asarray(inst).astype(np.int64)]
    return out


def kernel(mask, features, W, b, inst):
    in_maps, perm, groups = make_in_maps(mask, features, W, b, inst)
    nc = _get_program(groups)
    res = bass_utils.run_bass_kernel_spmd(nc, in_maps, core_ids=list(range(NCORES)))
    return postprocess(res.results, perm, b, inst)
